# revision 33
# baseline (speedup 1.0000x reference)
"""Trainium2 Bass kernel: 16-head MHA (B=2, T=2048, D=1024), head-TP over 8 cores.

Per core c: heads 2c, 2c+1 (128 channels). Device computes x@Wqkv(+b) for its
head slice, scoresT=K@Q^T (scale folded into Wq), exp via ACT, P@V with an
appended ones-column producing the softmax denominator for free, normalize,
then partial proj = attn_c @ Wproj[c-slice]. Host sums the 8 partials + b_proj
(+ bv@Wproj, since the V bias passes through softmax as a constant).

Steady state is jointly PE/ACT-limited (~1.06us/EXP on ACT; PE carries
scores+PV plus all qkv/proj work). Schedule structure:
 - one GLOBAL software pipeline over all (b, qc, kc): 2-kc groups of
   [scores pair, scores pair][4x K=128 PV][fillers]; PV lags the pairs by
   >=2 groups across qc boundaries (elastic queue, exp ring bufs=11), and
   each qc's normalization lands inside the next qc's group 1 — the PE
   never drains/refills at a qc switch
 - grouping 2 kcs halves the ~107ns penalty paid by the first matmul after
   every switch between packed-K64 mode and K=128 mode
 - score matmuls (K=64) run concurrently on PE row-groups via tile_position
 - qkv supply (kT/qT 512-token chains, V 2-chunk pairs) is a unit list with
   (deadline, earliest) per unit; a per-group load leveler (TARGET ~2.55us)
   fills PE slack uniformly instead of bunching supply in each batch's qc0.
   qk chains keep N=512: at N=256 the per-step LDWEIGHTS (~150ns) no longer
   hides under the stream and the chain runs 2x slow
 - DMA plan: critical path (wk, wq, x t0) first, descriptors spread over
   sync/gpsimd/scalar (each DIRECT2D costs ~640ns of queue time); b1's x
   arrives as one merged [128,2048] DMA per k-chunk
 - PE warm-up junk matmuls cover the initial DMA window (HAM clock ramp);
   more junk overlaps the final norm so the tail projs run at 2.4GHz
 - tail out-DMAs avoid the gpsimd queue (its ~5us teardown dge_drain delays
   a DMA issued right before context exit)
 - ACT does exp only: q/k bias adds live on DVE, V bias folded into host
   b_proj; PSUM->SBUF copies on DVE (GpSimd cannot read PSUM)
"""

import numpy as np
import ml_dtypes
from contextlib import ExitStack

B, T, C = 2, 2048, 1024
H, DH = 16, 64
NCORES = 8
CH = 128               # channels per core = 2 heads
NTOK = B * T           # 4096
NKC = T // 128         # 16 key chunks per batch
NQC = T // 512         # 4 query chunks per batch
SCALE = DH ** -0.5

_CACHE = {}


def _build():
    import concourse.bass as bass  # noqa: F401
    import concourse.bacc as bacc
    import concourse.mybir as mybir
    import concourse.tile as tile

    f32 = mybir.dt.float32
    bf16 = mybir.dt.bfloat16
    EXP = mybir.ActivationFunctionType.Exp

    # Bacc (not Bass): its compile() runs move_matmul_waits_to_ldweights +
    # generate_event_semaphores, without which walrus rejects matmuls
    # carrying 2 sync waits ("Too many sync wait commands").
    nc = bacc.Bacc("TRN2", target_bir_lowering=False, debug=False)
    xT_d = nc.declare_dram_parameter("xT", [C, NTOK], bf16, isOutput=False)
    wq_d = nc.declare_dram_parameter("wq", [128, C], bf16, isOutput=False)
    wk_d = nc.declare_dram_parameter("wk", [128, C], bf16, isOutput=False)
    wv_d = nc.declare_dram_parameter("wv", [128, C], bf16, isOutput=False)
    wp_d = nc.declare_dram_parameter("wp", [CH, C], bf16, isOutput=False)
    bqc_d = nc.declare_dram_parameter("bqc", [CH, 1], f32, isOutput=False)
    bkc_d = nc.declare_dram_parameter("bkc", [CH, 1], f32, isOutput=False)
    out_d = nc.declare_dram_parameter("out", [NTOK, C], bf16, isOutput=True)

    with tile.TileContext(nc) as tc, ExitStack() as ctx:
        ep = ctx.enter_context

        # ---------------- persistent SBUF ----------------
        xT_pool = ep(tc.tile_pool(name="xT", bufs=8))
        xT_sb = [xT_pool.tile([128, NTOK], bf16, name=f"xT{k}", tag="xT") for k in range(8)]
        w_pool = ep(tc.tile_pool(name="w", bufs=4))
        wq_sb = w_pool.tile([128, C], bf16, tag="wq")
        wk_sb = w_pool.tile([128, C], bf16, tag="wk")
        wv_sb = w_pool.tile([128, C], bf16, tag="wv")
        wp_sb = w_pool.tile([CH, C], bf16, tag="wp")
        b_pool = ep(tc.tile_pool(name="bias", bufs=1))
        bqc_sb = b_pool.tile([CH, 1], f32, tag="bqc")
        bkc_sb = b_pool.tile([CH, 1], f32, tag="bkc")
        warm_pool = ep(tc.tile_pool(name="warm", bufs=1))
        warm_sb = warm_pool.tile([128, 256], bf16, tag="warm")
        qk_pool = ep(tc.tile_pool(name="qk", bufs=2))
        qT_sb = qk_pool.tile([CH, NTOK], bf16, tag="qT")
        kT_sb = qk_pool.tile([CH, NTOK], bf16, tag="kT")
        v_pool = ep(tc.tile_pool(name="v", bufs=1))
        # per head: B*NKC chunks of [128 keys, 64 ones cols | 64 feats]; the
        # ones cols make the PV matmul replicate the softmax denominator onto
        # output partitions 0:64 for free (recip reads physical partition 0).
        # single 4D tile [128, chunk, head, col] (chunk-major so a paired
        # 2-chunk psum drain is one DVE copy); memset touches only ones cols.
        v_sb = v_pool.tile([128, B * NKC, 2, 128], bf16, name="v", tag="v")
        attn_pool = ep(tc.tile_pool(name="attn", bufs=2))
        attnT = [attn_pool.tile([CH, T], bf16, name=f"attnT{b}", tag="attnT") for b in range(B)]
        exp_pool = ep(tc.tile_pool(name="exp", bufs=13))
        bc_pool = ep(tc.tile_pool(name="bcsb", bufs=1))
        out_pool = ep(tc.tile_pool(name="outsb", bufs=4))

        # ---------------- load inputs ----------------
        # critical path to the first exp: wk/wq + x(t0) -> kT(0), qT(0) ->
        # first scores pair. Issue those DMAs first, spread over 4 queues
        # (each DIRECT2D descriptor costs ~640ns of queue-engine time).
        def xchunk(q, k, t):
            q.dma_start(
                xT_sb[k][:, t * 512:(t + 1) * 512],
                xT_d[k * 128:(k + 1) * 128, t * 512:(t + 1) * 512])

        # warm-up feed tile first: tiny gpsimd memset with no deps, so the
        # PE warm-up matmuls can start the moment the preamble ends
        nc.gpsimd.memset(warm_sb[:], 1.0)

        nc.gpsimd.dma_start(wk_sb[:], wk_d[:])
        nc.scalar.dma_start(wq_sb[:], wq_d[:])
        for k in range(8):
            xchunk((nc.sync, nc.gpsimd, nc.scalar)[k % 3], k, 0)
        nc.scalar.dma_start(bkc_sb[:], bkc_d[:])
        nc.scalar.dma_start(bqc_sb[:], bqc_d[:])
        for k in range(8):
            xchunk((nc.sync, nc.gpsimd, nc.scalar)[k % 3], k, 1)
        nc.scalar.dma_start(wv_sb[:], wv_d[:])
        # ones columns of the v tile (value cols 64:128 overwritten later);
        # on the DVE queue after its DMA issues, done before the first PV
        nc.vector.memset(v_sb[:, :, :, 0:64], 1.0)
        for t in range(2, 4):
            for k in range(8):
                q = nc.sync if k % 2 == 0 else nc.gpsimd
                xchunk(q, k, t)
        # b1's token slices (t4..7) aren't needed until ~G4: one merged
        # [128, 2048] DMA per k-chunk keeps descriptor-issue time low
        for k in range(8):
            q = nc.sync if k % 2 == 0 else nc.gpsimd
            q.dma_start(xT_sb[k][:, 4 * 512:8 * 512],
                        xT_d[k * 128:(k + 1) * 128, 4 * 512:8 * 512])
        nc.sync.dma_start(wp_sb[:], wp_d[:])

        # ---------------- PE warm-up ----------------
        # The HAM clock gate keeps PE at 1.2 GHz until ~3.4us of sustained
        # activity. Run junk matmuls on memset data during the initial DMA
        # window so the real matmuls start at 2.4 GHz.
        with tc.tile_pool(name="warm_ps", bufs=1, space="PSUM") as warm_psp:
            wps = warm_psp.tile([64, 256], f32, name="warm_ps", tag="warm_ps")
            for _ in range(12):
                nc.tensor.matmul(wps[:], lhsT=warm_sb[:, 0:64],
                                 rhs=warm_sb[:], start=True, stop=True)

        # ---------------- attention with JIT qkv ----------------
        with tc.tile_pool(name="scores_ps", bufs=2, space="PSUM") as scores_ps, \
             tc.tile_pool(name="pv_ps", bufs=2, space="PSUM") as pv_ps, \
             tc.tile_pool(name="proj_ps", bufs=2, space="PSUM") as proj_ps:

            def emit_qk_half(w_sb, bias_col, dst, ts):
                """[CH,512] q or k projection of 512-token slice ts (0..7).
                N=512 keeps the per-step LDWEIGHTS (~97ns) hidden under the
                213ns stream; smaller N goes LDW-bound and doubles the cost."""
                ps = proj_ps.tile([128, 512], f32, name="qk_ps", tag="pj")
                sl = slice(ts * 512, (ts + 1) * 512)
                for k in range(8):
                    nc.tensor.matmul(
                        ps[:], lhsT=w_sb[:, k * 128:(k + 1) * 128],
                        rhs=xT_sb[k][:, sl], start=(k == 0), stop=(k == 7))
                nc.vector.tensor_scalar_add(dst[:, sl], ps[:], bias_col[:])

            def emit_v_pair(tt):
                """two adjacent [128 tokens, 2x64] v chunks (tt, tt+1) into one
                psum tile: halves the pj-slot grabs and DVE drain count. The
                two accumulation chains are sequential so chunk 1's start=True
                bank-bit clear cannot corrupt chunk 0's finished values."""
                ps = proj_ps.tile([128, 2, 2, 64], f32, name="v_ps", tag="pj")
                for j in range(2):
                    for k in range(8):
                        nc.tensor.matmul(
                            ps[:, j], lhsT=xT_sb[k][:, (tt + j) * 128:(tt + j + 1) * 128],
                            rhs=wv_sb[:, k * 128:(k + 1) * 128],
                            start=(k == 0), stop=(k == 7), skip_group_check=True)
                nc.vector.tensor_copy(v_sb[:, tt:tt + 2, :, 64:128], ps[:])

            pending_proj = []

            def emit_proj_tc(b, tci, last=False):
                """proj partial for one 128-token chunk: out += attn @ Wp_c"""
                osb = out_pool.tile([128, 1024], bf16, name="out_sb", tag="out_sb")
                for ncol in range(2):
                    pps = proj_ps.tile([128, 512], f32, name="proj_ps", tag="pj")
                    nc.tensor.matmul(
                        pps[:],
                        lhsT=attnT[b][:, tci * 128:(tci + 1) * 128],
                        rhs=wp_sb[:, ncol * 512:(ncol + 1) * 512],
                        start=True, stop=True)
                    # ACT is idle at the very end (all exp done) — use it there
                    if last and ncol == 1:
                        nc.scalar.copy(osb[:, ncol * 512:(ncol + 1) * 512], pps[:])
                    else:
                        nc.vector.tensor_copy(
                            osb[:, ncol * 512:(ncol + 1) * 512], pps[:])
                if last:
                    # not gpsimd: its teardown dge_drain (~5us) runs early and
                    # a DMA issued on it right before context exit executes
                    # only after the drain completes
                    oq = (nc.scalar, nc.sync, nc.scalar, nc.sync)[tci % 4]
                else:
                    oq = nc.gpsimd if tci % 2 == 0 else nc.sync
                oq.dma_start(
                    out_d[b * T + tci * 128: b * T + (tci + 1) * 128, :], osb[:])

            # ---- filler units: (deadline_G, earliest_G, est_ns, fn) ----
            # The steady-state pacer is whichever engine each group loads
            # more; leveling the filler work across all 64 global groups
            # keeps both PE and ACT near-saturated. deadline = last group in
            # which the unit may be emitted and still land before its
            # consumer; earliest = first group whose inputs (x DMA waves)
            # have certainly arrived.
            units = []

            def add_unit(deadline, earliest, est, fn):
                units.append([deadline, earliest, est, fn])

            QK_NS, VP_NS = 1760, 930
            # kT halves 1..7 (0 is pre-phase): kT(t) first used by pairs of
            # kc=4t at group 2t (b0) / 32+2(t-4) (b1)
            for t, (dl, ea) in enumerate(
                    [(1, 0), (3, 1), (5, 2), (28, 4), (32, 5), (34, 6),
                     (36, 7)], start=1):
                add_unit(dl, ea, QK_NS,
                         lambda t=t: emit_qk_half(wk_sb, bkc_sb, kT_sb, t))
            # qT per (b, qc) except (0,0) (pre-phase); due before the qc's
            # first pair group
            for bq in range(1, 8):
                add_unit(bq * 8 - 2, max(0, bq * 8 - 10), QK_NS,
                         lambda bq=bq: emit_qk_half(wq_sb, bqc_sb, qT_sb, bq))
            # v pairs: chunks (c, c+1) due before PV(c) at qc-local group
            # c//2 + 2 (PV lags pairs by >=2 groups)
            for c in range(0, 16, 2):
                add_unit(c // 2 + 1, max(0, c // 2 - 3), VP_NS,
                         lambda c=c: emit_v_pair(c))
            for c in range(16, 32, 2):
                add_unit(32 + (c - 16) // 2 + 1, 8, VP_NS,
                         lambda c=c: emit_v_pair(c))
            units.sort(key=lambda u: u[0])

            # pre-phase: minimal deps for (b0,qc0) scores: kT(t0), qT(qc0)
            emit_qk_half(wk_sb, bkc_sb, kT_sb, 0)
            emit_qk_half(wq_sb, bqc_sb, qT_sb, 0)

            # ---- global software pipeline over all (b, qc, kc) ----
            # One continuous stream of 2-kc groups; PV lags the scores pairs
            # by 2 groups GLOBALLY (crossing qc boundaries), so the PE never
            # drains/refills at a qc switch. Each qc's normalization lands
            # inside the next qc's group 1, right after its last PV.
            pv_state = {}        # (b, qc) -> [pv0, pv1] psum tiles
            exp_store = {}       # (b, qc, kc) -> ex sbuf tile
            pv_queue = []        # (b, qc, kc) emitted scores awaiting PV

            def emit_scores(b, qc, kc):
                q_sl = slice(b * T + qc * 512, b * T + (qc + 1) * 512)
                sc = scores_ps.tile([128, 1024], f32, name="sc_ps", tag="ps")
                k_sl = slice(b * T + kc * 128, b * T + (kc + 1) * 128)
                # the two heads occupy PE row-groups 0-63 / 64-127 and
                # different PSUM banks -> they execute concurrently
                for h in range(2):
                    nc.tensor.matmul(
                        sc[:, h * 512:(h + 1) * 512],
                        lhsT=kT_sb[h * 64:(h + 1) * 64, k_sl],
                        rhs=qT_sb[h * 64:(h + 1) * 64, q_sl],
                        start=True, stop=True,
                        tile_position=(h * 64, 0))
                ex = exp_pool.tile([128, 1024], bf16, name="exp_sb", tag="exp_sb")
                nc.scalar.activation(ex[:], sc[:], EXP)
                exp_store[(b, qc, kc)] = ex
                pv_queue.append((b, qc, kc))

            def emit_pv(b, qc, kc):
                if (b, qc) not in pv_state:
                    pv_state[(b, qc)] = [
                        pv_ps.tile([128, 512], f32, name=f"pv{h}", tag="pv")
                        for h in range(2)]
                pv = pv_state[(b, qc)]
                ex = exp_store.pop((b, qc, kc))
                for h in range(2):
                    nc.tensor.matmul(
                        pv[h][:],
                        lhsT=v_sb[:, b * NKC + kc, h, :],
                        rhs=ex[:, h * 512:(h + 1) * 512],
                        start=(kc == 0), stop=(kc == NKC - 1),
                        skip_group_check=True)
                if kc == NKC - 1:
                    emit_norm(b, qc)

            def emit_norm(b, qc):
                # normalize: D replicated on pv partitions 0:64, PV on
                # 64:128. Per-head order (recip h, mul h) frees pv[h]'s
                # psum slot as early as possible for the next qc.
                pv = pv_state.pop((b, qc))
                bcsb = bc_pool.tile([64, 1024], f32, name="bc_sb", tag="bc_sb")
                qcs = slice(qc * 512, (qc + 1) * 512)
                for h in range(2):
                    nc.vector.reciprocal_approx_fast(
                        out=bcsb[:, h * 512:(h + 1) * 512],
                        in_=pv[h][0:64, :])
                    nc.vector.tensor_mul(
                        attnT[b][h * 64:(h + 1) * 64, qcs],
                        pv[h][64:128, :], bcsb[:, h * 512:(h + 1) * 512])
                pending_proj.extend(
                    (cur_G[0] + 2, b, tci) for tci in range(qc * 4, (qc + 1) * 4))

            cur_G = [0]
            TARGET = 2550
            for b in range(B):
                for qc in range(NQC):
                    for g in range(NKC // 2):
                        G = cur_G[0]
                        emit_scores(b, qc, 2 * g)
                        emit_scores(b, qc, 2 * g + 1)
                        load = 533
                        # deadline-forced supply units
                        while units and units[0][0] <= G:
                            u = units.pop(0)
                            u[3]()
                            load += u[2]
                        # PV debt: hard cap from the exp ring, else pay down
                        # while the group has budget
                        while len(pv_queue) > 10 or (
                                len(pv_queue) > 4 and load + 971 <= 2700):
                            emit_pv(*pv_queue.pop(0))
                            emit_pv(*pv_queue.pop(0))
                            load += 971
                        # proj drains keep the out-DMA pipeline moving; they
                        # must lag their norm by >=2 groups (attnT via DVE)
                        pops = 0
                        while (pending_proj and pending_proj[0][0] <= G
                               and pops < 2 and load + 430 <= TARGET + 300):
                            _, pb, ptci = pending_proj.pop(0)
                            emit_proj_tc(pb, ptci)
                            load += 430
                            pops += 1
                        # optional supply units to level the PE load
                        for u in list(units):
                            if load >= TARGET:
                                break
                            if u[1] <= G and load + u[2] <= TARGET + 350:
                                units.remove(u)
                                u[3]()
                                load += u[2]
                        cur_G[0] += 1

            # drain: last 4 PV units (b1,qc3 kc12..15) + final norm + projs.
            # Junk matmuls keep the HAM activity window full while the DVE
            # norm runs, so the proj burst goes at 2.4GHz.
            assert not units
            while pv_queue:
                emit_pv(*pv_queue.pop(0))
            for j in range(3):
                wps = proj_ps.tile([128, 512], f32, name="warmx", tag="pj")
                for _ in range(9):
                    nc.tensor.matmul(wps[:, 0:256],
                                     lhsT=warm_sb[:, 0:128],
                                     rhs=warm_sb[:, 0:256],
                                     start=True, stop=True,
                                     skip_group_check=True)
            for _, b, tci in pending_proj:
                emit_proj_tc(b, tci, last=True)

    nc.compile()
    return nc


def _prep_inputs(x, W_qkv, b_qkv, W_proj, b_proj):
    bf = ml_dtypes.bfloat16
    xT = np.ascontiguousarray(
        x.reshape(NTOK, C).T).astype(bf)
    in_maps = []
    for c in range(NCORES):
        cs = slice(c * CH, (c + 1) * CH)
        wq = np.ascontiguousarray(
            (W_qkv[:, c * CH:(c + 1) * CH] * SCALE)
            .reshape(8, 128, CH).transpose(1, 0, 2).reshape(128, C)).astype(bf)
        wk = np.ascontiguousarray(
            W_qkv[:, C + c * CH:C + (c + 1) * CH]
            .reshape(8, 128, CH).transpose(1, 0, 2).reshape(128, C)).astype(bf)
        wv = np.ascontiguousarray(
            W_qkv[:, 2 * C + c * CH:2 * C + (c + 1) * CH]
            .reshape(8, 128, CH).transpose(1, 0, 2).reshape(128, C)).astype(bf)
        wp = np.ascontiguousarray(W_proj[cs, :]).astype(bf)
        bqc = (b_qkv[c * CH:(c + 1) * CH] * SCALE).reshape(CH, 1).astype(np.float32)
        bkc = b_qkv[C + c * CH:C + (c + 1) * CH].reshape(CH, 1).astype(np.float32)
        in_maps.append({
            "xT": xT, "wq": wq, "wk": wk, "wv": wv, "wp": wp,
            "bqc": bqc, "bkc": bkc,
        })
    return in_maps


def _run(inputs, trace=False):
    from concourse import bass_utils
    if "nc" not in _CACHE:
        _CACHE["nc"] = _build()
    nc = _CACHE["nc"]
    x = np.asarray(inputs["x"], np.float32)
    W_qkv = np.asarray(inputs["W_qkv"], np.float32)
    b_qkv = np.asarray(inputs["b_qkv"], np.float32)
    W_proj = np.asarray(inputs["W_proj"], np.float32)
    b_proj = np.asarray(inputs["b_proj"], np.float32)
    in_maps = _prep_inputs(x, W_qkv, b_qkv, W_proj, b_proj)
    br = bass_utils.run_bass_kernel_spmd(
        nc, in_maps, core_ids=list(range(NCORES)), trace=trace)
    partial = np.zeros((NTOK, C), np.float64)
    for r in br.results:
        partial += np.asarray(r["out"]).astype(np.float64)
    # V bias passes through softmax (weights sum to 1) -> constant bv@Wp
    bias = b_proj.astype(np.float64) + (
        b_qkv[2 * C:].astype(np.float64) @ W_proj.astype(np.float64))
    out = (partial + bias[None, :]).astype(np.float32).reshape(B, T, C)
    return out, br


def kernel(**inputs) -> np.ndarray:
    out, _ = _run(inputs, trace=False)
    return out



# revision 34
# speedup vs baseline: 1.0105x; 1.0105x over previous
"""Trainium2 Bass kernel: 16-head MHA (B=2, T=2048, D=1024), head-TP over 8 cores.

Per core c: heads 2c, 2c+1 (128 channels). Device computes x@Wqkv(+b) for its
head slice, scoresT=K@Q^T (scale folded into Wq), exp via ACT, P@V with an
appended ones-column producing the softmax denominator for free, normalize,
then partial proj = attn_c @ Wproj[c-slice]. Host sums the 8 partials + b_proj
(+ bv@Wproj, since the V bias passes through softmax as a constant).

Steady state is jointly PE/ACT-limited (~1.06us/EXP on ACT; PE carries
scores+PV plus all qkv/proj work). Schedule structure:
 - one GLOBAL software pipeline over all (b, qc, kc): 2-kc groups of
   [scores pair, scores pair][4x K=128 PV][fillers]; PV lags the pairs by
   >=2 groups across qc boundaries (elastic queue, exp ring bufs=11), and
   each qc's normalization lands inside the next qc's group 1 — the PE
   never drains/refills at a qc switch
 - grouping 2 kcs halves the ~107ns penalty paid by the first matmul after
   every switch between packed-K64 mode and K=128 mode
 - score matmuls (K=64) run concurrently on PE row-groups via tile_position
 - qkv supply (kT/qT 512-token chains, V 2-chunk pairs) is a unit list with
   (deadline, earliest) per unit; a per-group load leveler (TARGET ~2.55us)
   fills PE slack uniformly instead of bunching supply in each batch's qc0.
   qk chains keep N=512: at N=256 the per-step LDWEIGHTS (~150ns) no longer
   hides under the stream and the chain runs 2x slow
 - DMA plan: critical path (wk, wq, x t0) first, descriptors spread over
   sync/gpsimd/scalar (each DIRECT2D costs ~640ns of queue time); b1's x
   arrives as one merged [128,2048] DMA per k-chunk
 - PE warm-up junk matmuls cover the initial DMA window (HAM clock ramp);
   more junk overlaps the final norm so the tail projs run at 2.4GHz
 - tail out-DMAs avoid the gpsimd queue (its ~5us teardown dge_drain delays
   a DMA issued right before context exit)
 - ACT does exp only: q/k bias adds live on DVE, V bias folded into host
   b_proj; PSUM->SBUF copies on DVE (GpSimd cannot read PSUM)
"""

import numpy as np
import ml_dtypes
from contextlib import ExitStack

B, T, C = 2, 2048, 1024
H, DH = 16, 64
NCORES = 8
CH = 128               # channels per core = 2 heads
NTOK = B * T           # 4096
NKC = T // 128         # 16 key chunks per batch
NQC = T // 512         # 4 query chunks per batch
SCALE = DH ** -0.5

_CACHE = {}


def _build():
    import concourse.bass as bass  # noqa: F401
    import concourse.bacc as bacc
    import concourse.mybir as mybir
    import concourse.tile as tile

    f32 = mybir.dt.float32
    bf16 = mybir.dt.bfloat16
    EXP = mybir.ActivationFunctionType.Exp

    # Bacc (not Bass): its compile() runs move_matmul_waits_to_ldweights +
    # generate_event_semaphores, without which walrus rejects matmuls
    # carrying 2 sync waits ("Too many sync wait commands").
    nc = bacc.Bacc("TRN2", target_bir_lowering=False, debug=False)
    xT_d = nc.declare_dram_parameter("xT", [C, NTOK], bf16, isOutput=False)
    wq_d = nc.declare_dram_parameter("wq", [128, C], bf16, isOutput=False)
    wk_d = nc.declare_dram_parameter("wk", [128, C], bf16, isOutput=False)
    wv_d = nc.declare_dram_parameter("wv", [128, C], bf16, isOutput=False)
    wp_d = nc.declare_dram_parameter("wp", [CH, C], bf16, isOutput=False)
    bqc_d = nc.declare_dram_parameter("bqc", [CH, 1], f32, isOutput=False)
    bkc_d = nc.declare_dram_parameter("bkc", [CH, 1], f32, isOutput=False)
    out_d = nc.declare_dram_parameter("out", [NTOK, C], bf16, isOutput=True)

    with tile.TileContext(nc) as tc, ExitStack() as ctx:
        ep = ctx.enter_context

        # ---------------- persistent SBUF ----------------
        xT_pool = ep(tc.tile_pool(name="xT", bufs=8))
        xT_sb = [xT_pool.tile([128, NTOK], bf16, name=f"xT{k}", tag="xT") for k in range(8)]
        w_pool = ep(tc.tile_pool(name="w", bufs=4))
        wq_sb = w_pool.tile([128, C], bf16, tag="wq")
        wk_sb = w_pool.tile([128, C], bf16, tag="wk")
        wv_sb = w_pool.tile([128, C], bf16, tag="wv")
        wp_sb = w_pool.tile([CH, C], bf16, tag="wp")
        b_pool = ep(tc.tile_pool(name="bias", bufs=1))
        bqc_sb = b_pool.tile([CH, 1], f32, tag="bqc")
        bkc_sb = b_pool.tile([CH, 1], f32, tag="bkc")
        warm_pool = ep(tc.tile_pool(name="warm", bufs=1))
        warm_sb = warm_pool.tile([128, 256], bf16, tag="warm")
        qk_pool = ep(tc.tile_pool(name="qk", bufs=2))
        qT_sb = qk_pool.tile([CH, NTOK], bf16, tag="qT")
        kT_sb = qk_pool.tile([CH, NTOK], bf16, tag="kT")
        v_pool = ep(tc.tile_pool(name="v", bufs=1))
        # per head: B*NKC chunks of [128 keys, 64 ones cols | 64 feats]; the
        # ones cols make the PV matmul replicate the softmax denominator onto
        # output partitions 0:64 for free (recip reads physical partition 0).
        # single 4D tile [128, chunk, head, col] (chunk-major so a paired
        # 2-chunk psum drain is one DVE copy); memset touches only ones cols.
        v_sb = v_pool.tile([128, B * NKC, 2, 128], bf16, name="v", tag="v")
        attn_pool = ep(tc.tile_pool(name="attn", bufs=2))
        attnT = [attn_pool.tile([CH, T], bf16, name=f"attnT{b}", tag="attnT") for b in range(B)]
        exp_pool = ep(tc.tile_pool(name="exp", bufs=11))
        bc_pool = ep(tc.tile_pool(name="bcsb", bufs=1))
        out_pool = ep(tc.tile_pool(name="outsb", bufs=4))

        # ---------------- load inputs ----------------
        # critical path to the first exp: wk/wq + x(t0) -> kT(0), qT(0) ->
        # first scores pair. Issue those DMAs first, spread over 4 queues
        # (each DIRECT2D descriptor costs ~640ns of queue-engine time).
        def xchunk(q, k, t):
            q.dma_start(
                xT_sb[k][:, t * 512:(t + 1) * 512],
                xT_d[k * 128:(k + 1) * 128, t * 512:(t + 1) * 512])

        # warm-up feed tile first: tiny gpsimd memset with no deps, so the
        # PE warm-up matmuls can start the moment the preamble ends
        nc.gpsimd.memset(warm_sb[:], 1.0)

        nc.gpsimd.dma_start(wk_sb[:], wk_d[:])
        nc.scalar.dma_start(wq_sb[:], wq_d[:])
        for k in range(8):
            xchunk((nc.sync, nc.gpsimd, nc.scalar)[k % 3], k, 0)
        nc.scalar.dma_start(bkc_sb[:], bkc_d[:])
        nc.scalar.dma_start(bqc_sb[:], bqc_d[:])
        for k in range(8):
            xchunk((nc.sync, nc.gpsimd, nc.scalar)[k % 3], k, 1)
        nc.scalar.dma_start(wv_sb[:], wv_d[:])
        # ones columns of the v tile (value cols 64:128 overwritten later);
        # on the DVE queue after its DMA issues, done before the first PV
        nc.vector.memset(v_sb[:, :, :, 0:64], 1.0)
        for t in range(2, 4):
            for k in range(8):
                q = nc.sync if k % 2 == 0 else nc.gpsimd
                xchunk(q, k, t)
        # b1's token slices (t4..7) aren't needed until ~G4: one merged
        # [128, 2048] DMA per k-chunk keeps descriptor-issue time low
        for k in range(8):
            q = nc.sync if k % 2 == 0 else nc.gpsimd
            q.dma_start(xT_sb[k][:, 4 * 512:8 * 512],
                        xT_d[k * 128:(k + 1) * 128, 4 * 512:8 * 512])
        nc.sync.dma_start(wp_sb[:], wp_d[:])

        # ---------------- PE warm-up ----------------
        # The HAM clock gate keeps PE at 1.2 GHz until ~3.4us of sustained
        # activity. Run junk matmuls on memset data during the initial DMA
        # window so the real matmuls start at 2.4 GHz.
        with tc.tile_pool(name="warm_ps", bufs=1, space="PSUM") as warm_psp:
            wps = warm_psp.tile([64, 256], f32, name="warm_ps", tag="warm_ps")
            for _ in range(12):
                nc.tensor.matmul(wps[:], lhsT=warm_sb[:, 0:64],
                                 rhs=warm_sb[:], start=True, stop=True)

        # ---------------- attention with JIT qkv ----------------
        with tc.tile_pool(name="scores_ps", bufs=2, space="PSUM") as scores_ps, \
             tc.tile_pool(name="pv_ps", bufs=2, space="PSUM") as pv_ps, \
             tc.tile_pool(name="proj_ps", bufs=2, space="PSUM") as proj_ps:

            def emit_qk_half(w_sb, bias_col, dst, ts):
                """[CH,512] q or k projection of 512-token slice ts (0..7).
                N=512 keeps the per-step LDWEIGHTS (~97ns) hidden under the
                213ns stream; smaller N goes LDW-bound and doubles the cost."""
                ps = proj_ps.tile([128, 512], f32, name="qk_ps", tag="pj")
                sl = slice(ts * 512, (ts + 1) * 512)
                for k in range(8):
                    nc.tensor.matmul(
                        ps[:], lhsT=w_sb[:, k * 128:(k + 1) * 128],
                        rhs=xT_sb[k][:, sl], start=(k == 0), stop=(k == 7))
                nc.vector.tensor_scalar_add(dst[:, sl], ps[:], bias_col[:])

            def emit_v_pair(tt):
                """two adjacent [128 tokens, 2x64] v chunks (tt, tt+1) into one
                psum tile: halves the pj-slot grabs and DVE drain count. The
                two accumulation chains are sequential so chunk 1's start=True
                bank-bit clear cannot corrupt chunk 0's finished values."""
                ps = proj_ps.tile([128, 2, 2, 64], f32, name="v_ps", tag="pj")
                for j in range(2):
                    for k in range(8):
                        nc.tensor.matmul(
                            ps[:, j], lhsT=xT_sb[k][:, (tt + j) * 128:(tt + j + 1) * 128],
                            rhs=wv_sb[:, k * 128:(k + 1) * 128],
                            start=(k == 0), stop=(k == 7), skip_group_check=True)
                nc.vector.tensor_copy(v_sb[:, tt:tt + 2, :, 64:128], ps[:])

            pending_proj = []

            def emit_proj_tc(b, tci, last=False):
                """proj partial for one 128-token chunk: out += attn @ Wp_c"""
                osb = out_pool.tile([128, 1024], bf16, name="out_sb", tag="out_sb")
                for ncol in range(2):
                    pps = proj_ps.tile([128, 512], f32, name="proj_ps", tag="pj")
                    nc.tensor.matmul(
                        pps[:],
                        lhsT=attnT[b][:, tci * 128:(tci + 1) * 128],
                        rhs=wp_sb[:, ncol * 512:(ncol + 1) * 512],
                        start=True, stop=True)
                    # ACT is idle at the very end (all exp done) — use it there
                    if last and ncol == 1:
                        nc.scalar.copy(osb[:, ncol * 512:(ncol + 1) * 512], pps[:])
                    else:
                        nc.vector.tensor_copy(
                            osb[:, ncol * 512:(ncol + 1) * 512], pps[:])
                if last:
                    # not gpsimd: its teardown dge_drain (~5us) runs early and
                    # a DMA issued on it right before context exit executes
                    # only after the drain completes
                    oq = (nc.scalar, nc.sync, nc.scalar, nc.sync)[tci % 4]
                else:
                    oq = nc.gpsimd if tci % 2 == 0 else nc.sync
                oq.dma_start(
                    out_d[b * T + tci * 128: b * T + (tci + 1) * 128, :], osb[:])

            # ---- filler units: (deadline_G, earliest_G, est_ns, fn) ----
            # The steady-state pacer is whichever engine each group loads
            # more; leveling the filler work across all 64 global groups
            # keeps both PE and ACT near-saturated. deadline = last group in
            # which the unit may be emitted and still land before its
            # consumer; earliest = first group whose inputs (x DMA waves)
            # have certainly arrived.
            units = []

            def add_unit(deadline, earliest, est, fn):
                units.append([deadline, earliest, est, fn])

            QK_NS, VP_NS = 1760, 930
            # kT halves 1..7 (0 is pre-phase): kT(t) first used by pairs of
            # kc=4t at group 2t (b0) / 32+2(t-4) (b1)
            for t, (dl, ea) in enumerate(
                    [(1, 0), (3, 1), (5, 2), (28, 4), (32, 5), (34, 6),
                     (36, 7)], start=1):
                add_unit(dl, ea, QK_NS,
                         lambda t=t: emit_qk_half(wk_sb, bkc_sb, kT_sb, t))
            # qT per (b, qc) except (0,0) (pre-phase); due before the qc's
            # first pair group
            for bq in range(1, 8):
                add_unit(bq * 8 - 2, max(0, bq * 8 - 10), QK_NS,
                         lambda bq=bq: emit_qk_half(wq_sb, bqc_sb, qT_sb, bq))
            # v pairs: chunks (c, c+1) due before PV(c) at qc-local group
            # c//2 + 2 (PV lags pairs by >=2 groups)
            for c in range(0, 16, 2):
                add_unit(c // 2 + 1, max(0, c // 2 - 3), VP_NS,
                         lambda c=c: emit_v_pair(c))
            for c in range(16, 32, 2):
                add_unit(32 + (c - 16) // 2 + 1, 8, VP_NS,
                         lambda c=c: emit_v_pair(c))
            units.sort(key=lambda u: u[0])

            # pre-phase: minimal deps for (b0,qc0) scores: kT(t0), qT(qc0)
            emit_qk_half(wk_sb, bkc_sb, kT_sb, 0)
            emit_qk_half(wq_sb, bqc_sb, qT_sb, 0)

            # ---- global software pipeline over all (b, qc, kc) ----
            # One continuous stream of 2-kc groups; PV lags the scores pairs
            # by 2 groups GLOBALLY (crossing qc boundaries), so the PE never
            # drains/refills at a qc switch. Each qc's normalization lands
            # inside the next qc's group 1, right after its last PV.
            pv_state = {}        # (b, qc) -> [pv0, pv1] psum tiles
            exp_store = {}       # (b, qc, kc) -> ex sbuf tile
            pv_queue = []        # (b, qc, kc) emitted scores awaiting PV

            def emit_scores(b, qc, kc):
                q_sl = slice(b * T + qc * 512, b * T + (qc + 1) * 512)
                sc = scores_ps.tile([128, 1024], f32, name="sc_ps", tag="ps")
                k_sl = slice(b * T + kc * 128, b * T + (kc + 1) * 128)
                # the two heads occupy PE row-groups 0-63 / 64-127 and
                # different PSUM banks -> they execute concurrently
                for h in range(2):
                    nc.tensor.matmul(
                        sc[:, h * 512:(h + 1) * 512],
                        lhsT=kT_sb[h * 64:(h + 1) * 64, k_sl],
                        rhs=qT_sb[h * 64:(h + 1) * 64, q_sl],
                        start=True, stop=True,
                        tile_position=(h * 64, 0))
                ex = exp_pool.tile([128, 1024], bf16, name="exp_sb", tag="exp_sb")
                nc.scalar.activation(ex[:], sc[:], EXP)
                exp_store[(b, qc, kc)] = ex
                pv_queue.append((b, qc, kc))

            def emit_pv(b, qc, kc):
                if (b, qc) not in pv_state:
                    pv_state[(b, qc)] = [
                        pv_ps.tile([128, 512], f32, name=f"pv{h}", tag="pv")
                        for h in range(2)]
                pv = pv_state[(b, qc)]
                ex = exp_store.pop((b, qc, kc))
                for h in range(2):
                    nc.tensor.matmul(
                        pv[h][:],
                        lhsT=v_sb[:, b * NKC + kc, h, :],
                        rhs=ex[:, h * 512:(h + 1) * 512],
                        start=(kc == 0), stop=(kc == NKC - 1),
                        skip_group_check=True)
                if kc == NKC - 1:
                    emit_norm(b, qc)

            def emit_norm(b, qc):
                # normalize: D replicated on pv partitions 0:64, PV on
                # 64:128. Per-head order (recip h, mul h) frees pv[h]'s
                # psum slot as early as possible for the next qc.
                pv = pv_state.pop((b, qc))
                bcsb = bc_pool.tile([64, 1024], f32, name="bc_sb", tag="bc_sb")
                qcs = slice(qc * 512, (qc + 1) * 512)
                for h in range(2):
                    nc.vector.reciprocal_approx_fast(
                        out=bcsb[:, h * 512:(h + 1) * 512],
                        in_=pv[h][0:64, :])
                    nc.vector.tensor_mul(
                        attnT[b][h * 64:(h + 1) * 64, qcs],
                        pv[h][64:128, :], bcsb[:, h * 512:(h + 1) * 512])
                pending_proj.extend(
                    (cur_G[0] + 2, b, tci) for tci in range(qc * 4, (qc + 1) * 4))

            cur_G = [0]
            TARGET = 2550
            for b in range(B):
                for qc in range(NQC):
                    for g in range(NKC // 2):
                        G = cur_G[0]
                        emit_scores(b, qc, 2 * g)
                        emit_scores(b, qc, 2 * g + 1)
                        load = 533
                        # deadline-forced supply units
                        while units and units[0][0] <= G:
                            u = units.pop(0)
                            u[3]()
                            load += u[2]
                        # PV debt: hard cap from the exp ring, else pay down
                        # while the group has budget
                        while len(pv_queue) > 8 or (
                                len(pv_queue) > 4 and load + 971 <= 2700):
                            emit_pv(*pv_queue.pop(0))
                            emit_pv(*pv_queue.pop(0))
                            load += 971
                        # proj drains keep the out-DMA pipeline moving; they
                        # must lag their norm by >=2 groups (attnT via DVE)
                        pops = 0
                        while (pending_proj and pending_proj[0][0] <= G
                               and pops < 2 and load + 430 <= TARGET + 300):
                            _, pb, ptci = pending_proj.pop(0)
                            emit_proj_tc(pb, ptci)
                            load += 430
                            pops += 1
                        # optional supply units to level the PE load
                        for u in list(units):
                            if load >= TARGET:
                                break
                            if u[1] <= G and load + u[2] <= TARGET + 350:
                                units.remove(u)
                                u[3]()
                                load += u[2]
                        cur_G[0] += 1

            # drain: last 4 PV units (b1,qc3 kc12..15) + final norm + projs.
            # Junk matmuls keep the HAM activity window full while the DVE
            # norm runs, so the proj burst goes at 2.4GHz.
            assert not units
            while pv_queue:
                emit_pv(*pv_queue.pop(0))
            for j in range(3):
                wps = proj_ps.tile([128, 512], f32, name="warmx", tag="pj")
                for _ in range(9):
                    nc.tensor.matmul(wps[:, 0:256],
                                     lhsT=warm_sb[:, 0:128],
                                     rhs=warm_sb[:, 0:256],
                                     start=True, stop=True,
                                     skip_group_check=True)
            for _, b, tci in pending_proj:
                emit_proj_tc(b, tci, last=True)

    nc.compile()
    return nc


def _prep_inputs(x, W_qkv, b_qkv, W_proj, b_proj):
    bf = ml_dtypes.bfloat16
    xT = np.ascontiguousarray(
        x.reshape(NTOK, C).T).astype(bf)
    in_maps = []
    for c in range(NCORES):
        cs = slice(c * CH, (c + 1) * CH)
        wq = np.ascontiguousarray(
            (W_qkv[:, c * CH:(c + 1) * CH] * SCALE)
            .reshape(8, 128, CH).transpose(1, 0, 2).reshape(128, C)).astype(bf)
        wk = np.ascontiguousarray(
            W_qkv[:, C + c * CH:C + (c + 1) * CH]
            .reshape(8, 128, CH).transpose(1, 0, 2).reshape(128, C)).astype(bf)
        wv = np.ascontiguousarray(
            W_qkv[:, 2 * C + c * CH:2 * C + (c + 1) * CH]
            .reshape(8, 128, CH).transpose(1, 0, 2).reshape(128, C)).astype(bf)
        wp = np.ascontiguousarray(W_proj[cs, :]).astype(bf)
        bqc = (b_qkv[c * CH:(c + 1) * CH] * SCALE).reshape(CH, 1).astype(np.float32)
        bkc = b_qkv[C + c * CH:C + (c + 1) * CH].reshape(CH, 1).astype(np.float32)
        in_maps.append({
            "xT": xT, "wq": wq, "wk": wk, "wv": wv, "wp": wp,
            "bqc": bqc, "bkc": bkc,
        })
    return in_maps


def _run(inputs, trace=False):
    from concourse import bass_utils
    if "nc" not in _CACHE:
        _CACHE["nc"] = _build()
    nc = _CACHE["nc"]
    x = np.asarray(inputs["x"], np.float32)
    W_qkv = np.asarray(inputs["W_qkv"], np.float32)
    b_qkv = np.asarray(inputs["b_qkv"], np.float32)
    W_proj = np.asarray(inputs["W_proj"], np.float32)
    b_proj = np.asarray(inputs["b_proj"], np.float32)
    in_maps = _prep_inputs(x, W_qkv, b_qkv, W_proj, b_proj)
    br = bass_utils.run_bass_kernel_spmd(
        nc, in_maps, core_ids=list(range(NCORES)), trace=trace)
    partial = np.zeros((NTOK, C), np.float64)
    for r in br.results:
        partial += np.asarray(r["out"]).astype(np.float64)
    # V bias passes through softmax (weights sum to 1) -> constant bv@Wp
    bias = b_proj.astype(np.float64) + (
        b_qkv[2 * C:].astype(np.float64) @ W_proj.astype(np.float64))
    out = (partial + bias[None, :]).astype(np.float32).reshape(B, T, C)
    return out, br


def kernel(**inputs) -> np.ndarray:
    out, _ = _run(inputs, trace=False)
    return out

